# revision 1
# baseline (speedup 1.0000x reference)
"""MLA (multi-head latent attention) kernel for Trainium2, 8-core SPMD.

Strategy (tensor-parallel over heads + token-parallel prologue):
  stage 1 : each core computes the fused down-projection for its 1/8 token
            slice (transposed layout: features on partitions), applies
            RMSNorm (q_c, kv_c) and RoPE (k_pe)  -> AllGather #1
  phase B : each core up-projects q/k/v for its 4 heads over all tokens and
            runs causal attention (scores kept transposed: keys on
            partitions, softmax denominator via ones-matmul), producing
            ctx^T per head  -> AllGather #2 (per token block)
  phase C : each core computes a 512-column slice of the output projection
            from the gathered ctx^T.

Precision: PSUM accumulation and softmax statistics are fp32; the RMSNorm
statistics pipeline runs float32r (1-pass fp22 matmuls). The data planes
(stage-1 GEMM, gathered latents, attention operands, ctx/output GEMM) run
bf16, which halves DMA and collective wire bytes at ~4e-3 end-to-end
relative error. Collective layout: one AllGather for the normalized latent
(8.7 MB bf16) + four per-token-block AllGathers for ctx^T (4.2 MB bf16
each), emission-interleaved with the output projection for overlap.

Measured on 8 axon trn2 cores: ~0.72 ms/iteration steady-state
(replication-slope method), rel err ~5.5e-3 vs the fp32 reference.
"""

import numpy as np

import concourse.bass as bass
import concourse.bacc as bacc
import concourse.mybir as mybir
import concourse.tile as tile
from concourse.bass_utils import run_bass_kernel_spmd

F32 = mybir.dt.float32
F32R = mybir.dt.float32r
BF16 = mybir.dt.bfloat16
AF = mybir.ActivationFunctionType

# ---- model dims (hardcoded per problem spec) ----
B, S, D = 2, 1024, 4096
H, QL, KVL = 32, 1536, 512
NOPE, ROPE, VD = 128, 64, 128
QKD = NOPE + ROPE
ROPE_BASE = 10000.0
EPS = 1e-6
NC = 8

DEFAULT_CFG = dict(B=B, S=S, D=D, H=H, QL=QL, KVL=KVL,
                   NOPE=NOPE, ROPE=ROPE, VD=VD, NC=NC,
                   bf16_s1=True, bf16_c=True, bf16_b=True)


def _rope_rotate(nc, pool, out64, in64, cos_sb, sin_sb, width, tag):
    """NeoX half-rotation on a [64, width] tile (in64 may be PSUM).

    out[0:32]  = x1*cos - x2*sin ; out[32:64] = x1*sin + x2*cos
    All ALU inputs are staged to partition-base-0 tiles first (DVE crossbar
    only supports quadrant-aligned cross-partition moves for single-src ops).
    """
    half = 32
    x1 = pool.tile([half, width], F32, name=f"{tag}_x1", tag="rope_x", bufs=2)
    x2 = pool.tile([half, width], F32, name=f"{tag}_x2", tag="rope_x", bufs=2)
    nc.vector.tensor_copy(x1[:], in64[0:half, :])
    nc.vector.tensor_copy(x2[:], in64[half:2 * half, :])
    t1 = pool.tile([half, width], F32, name=f"{tag}_t1", tag="rope_t", bufs=2)
    t2 = pool.tile([half, width], F32, name=f"{tag}_t2", tag="rope_t", bufs=2)
    nc.vector.tensor_mul(t1[:], x1[:], cos_sb)
    nc.vector.tensor_mul(t2[:], x2[:], sin_sb)
    nc.vector.tensor_sub(out64[0:half, :], t1[:], t2[:])
    t3 = pool.tile([half, width], F32, name=f"{tag}_t3", tag="rope_t", bufs=2)
    t4 = pool.tile([half, width], F32, name=f"{tag}_t4", tag="rope_t", bufs=2)
    nc.vector.tensor_mul(t3[:], x1[:], sin_sb)
    nc.vector.tensor_mul(t4[:], x2[:], cos_sb)
    nc.vector.tensor_add(out64[half:2 * half, :], t3[:], t4[:])


def build_program(cfg=None, reps=1, tiny_out=False, no_cc=False):
    c = dict(DEFAULT_CFG)
    if cfg:
        c.update(cfg)
    cB, cS, cD = c["B"], c["S"], c["D"]
    cQL, cKVL, cH = c["QL"], c["KVL"], c["H"]
    cNOPE, cROPE, cVD, cNC = c["NOPE"], c["ROPE"], c["VD"], c["NC"]
    S1DT = BF16 if c["bf16_s1"] else F32
    S1MMDT = BF16 if c["bf16_s1"] else F32R
    CDT = BF16 if c["bf16_c"] else F32
    CMMDT = BF16 if c["bf16_c"] else F32R
    BDT = BF16 if c["bf16_b"] else F32
    BMMDT = BF16 if c["bf16_b"] else F32R

    P = 128
    T = cB * cS
    TPC = T // cNC                  # stage-1 tokens per core
    HPC = cH // cNC                 # heads per core
    DPC = cD // cNC                 # output cols per core
    FSUM = cQL + cKVL + cROPE
    NQ, NKV = cQL // P, cKVL // P
    NFC = NQ + NKV                  # full-width fused chunks
    DCN = cD // P
    RH = cROPE // 2                 # 32
    TBLK = min(512, cS)             # phase-B token block
    NT = cS // TBLK
    KB = TBLK // P                  # key chunks per block
    NREG = TBLK // TPC              # stage-1 rank regions per block
    NROC = (HPC * cROPE + P - 1) // P   # q-rope out chunks
    SCALE = 1.0 / float(np.sqrt(np.float32(cNOPE + cROPE)))

    GWMAX = max(cKVL + cROPE, 4 * P)
    assert cS % TBLK == 0 and TBLK % P == 0 and TPC <= TBLK
    assert TBLK % TPC == 0 and cROPE == 64 and cNOPE == P and cVD == P

    nc = bacc.Bacc("TRN2", target_bir_lowering=False, debug=False,
                   num_devices=cNC)

    # ---- kernel I/O ----
    hiddenT_d = nc.dram_tensor("hiddenT", [cD, TPC], S1DT,
                               kind="ExternalInput")
    wfa_d = nc.dram_tensor("wfa", [cD, FSUM], S1DT, kind="ExternalInput")
    wqn_d = nc.dram_tensor("wqn", [cQL, HPC * cNOPE], BDT,
                           kind="ExternalInput")
    wqr_d = nc.dram_tensor("wqr", [cQL, HPC * cROPE], BDT,
                           kind="ExternalInput")
    wkn_d = nc.dram_tensor("wkn", [cKVL, HPC * cNOPE], BDT,
                           kind="ExternalInput")
    wv_d = nc.dram_tensor("wv", [cKVL, HPC * cVD], BDT, kind="ExternalInput")
    wo_d = nc.dram_tensor("wo", [cH * cVD, DPC], CDT, kind="ExternalInput")
    cosq_d = nc.dram_tensor("cosq", [RH, cS], F32, kind="ExternalInput")
    sinq_d = nc.dram_tensor("sinq", [RH, cS], F32, kind="ExternalInput")
    cosk_d = nc.dram_tensor("cosk", [RH, TPC], F32, kind="ExternalInput")
    sink_d = nc.dram_tensor("sink", [RH, TPC], F32, kind="ExternalInput")
    masks_d = nc.dram_tensor("masks", [P, KB, TBLK], BDT,
                             kind="ExternalInput")
    ones_d = nc.dram_tensor("ones", [P, P], F32, kind="ExternalInput")
    onesb_d = nc.dram_tensor("onesb", [P, P], BF16, kind="ExternalInput")
    if tiny_out:
        outT_d = nc.dram_tensor("outT_scratch", [DPC, T], F32)
        tick_d = nc.dram_tensor("tick", [P, 4], F32, kind="ExternalOutput")
    else:
        outT_d = nc.dram_tensor("outT", [DPC, T], F32, kind="ExternalOutput")
        tick_d = None

    RG = [list(range(cNC))]

    # stage-1 fused chunk grouping: chunks of 128 cols; rope (64) rides in
    # the last group.
    groups = [list(range(i, min(i + 4, NFC))) for i in range(0, NFC, 4)]
    groups[-1].append(NFC)

    def chunk_cols(fc):
        return (fc * P, P) if fc < NFC else (NFC * P, cROPE)

    with tile.TileContext(nc) as tc:
        with (
            tc.tile_pool(name="const", bufs=1) as cst,
            tc.tile_pool(name="dram", bufs=1, space="DRAM") as drp,
        ):
            # ---------- long-lived constants & weights ----------
            ones_sq = cst.tile([P, P], F32R, name="ones_sq")
            nc.sync.dma_start(ones_sq[:], ones_d[:, :].bitcast(F32R))
            ones_b = cst.tile([P, P], BMMDT, name="ones_b")
            if BDT == F32:
                nc.sync.dma_start(ones_b[:], ones_d[:, :].bitcast(F32R))
            else:
                nc.sync.dma_start(ones_b[:], onesb_d[:, :])
            eps_t = cst.tile([P, 1], F32, name="eps_t")
            nc.vector.memset(eps_t[:], EPS)

            masks_sb = []
            for m in range(KB):
                mt = cst.tile([P, TBLK], BMMDT, name=f"mask{m}")
                src_ap = masks_d[:, m, :]
                if BDT == F32:
                    src_ap = src_ap.bitcast(F32R)
                nc.sync.dma_start(mt[:], src_ap)
                masks_sb.append(mt)
            cosq_sb = cst.tile([RH, cS], F32, name="cosq")
            sinq_sb = cst.tile([RH, cS], F32, name="sinq")
            cosk_sb = cst.tile([RH, TPC], F32, name="cosk")
            sink_sb = cst.tile([RH, TPC], F32, name="sink")
            nc.sync.dma_start(cosq_sb[:], cosq_d[:, :])
            nc.sync.dma_start(sinq_sb[:], sinq_d[:, :])
            nc.sync.dma_start(cosk_sb[:], cosk_d[:, :])
            nc.sync.dma_start(sink_sb[:], sink_d[:, :])

            def bload(pool, dram, rows, cols, nm):
                t = pool.tile([rows, cols], BMMDT, name=nm)
                src_ap = dram
                if BDT == F32:
                    src_ap = src_ap.bitcast(F32R)
                nc.sync.dma_start(t[:], src_ap)
                return t

            wqn_sb, wqr_sb, wkn_sb, wv_sb = [], [], [], []
            for cc in range(NQ):
                wqn_sb.append(bload(cst, wqn_d[cc * P:(cc + 1) * P, :], P,
                                    HPC * cNOPE, f"wqn{cc}"))
                wqr_sb.append(bload(cst, wqr_d[cc * P:(cc + 1) * P, :], P,
                                    HPC * cROPE, f"wqr{cc}"))
            for cc in range(NKV):
                wkn_sb.append(bload(cst, wkn_d[cc * P:(cc + 1) * P, :], P,
                                    HPC * cNOPE, f"wkn{cc}"))
                wv_sb.append(bload(cst, wv_d[cc * P:(cc + 1) * P, :], P,
                                    HPC * cVD, f"wv{cc}"))

            def one_rep():
                ag1_in = drp.tile([FSUM, TPC], BDT, name="ag1_in")
                ag1_out = drp.tile([cNC, FSUM, TPC], BDT, name="ag1_out",
                                   addr_space="Shared")
                ag2_in = {}
                ag2_out = {}
                for b in range(cB):
                    for t_ in range(NT):
                        ag2_in[(b, t_)] = drp.tile(
                            [HPC * cVD, TBLK], CDT, name=f"ag2_in_{b}_{t_}")
                        ag2_out[(b, t_)] = drp.tile(
                            [cNC, HPC * cVD, TBLK], CDT,
                            name=f"ag2_out_{b}_{t_}", addr_space="Shared")
                # ---------- stage 1: fusedT = wfa^T @ hiddenT, norm, rope ----------
                with (
                    tc.tile_pool(name="s1", bufs=1) as s1,
                    tc.tile_pool(name="s1ps", bufs=1, space="PSUM") as s1ps,
                ):
                    ht_sb = []
                    for dc in range(DCN):
                        ht = s1.tile([P, TPC], S1MMDT, name=f"ht_{dc}",
                                     tag="htr", bufs=DCN)
                        src_ap = hiddenT_d[dc * P:(dc + 1) * P, :]
                        if S1DT == F32:
                            src_ap = src_ap.bitcast(F32R)
                        nc.sync.dma_start(ht[:], src_ap)
                        ht_sb.append(ht)
                    ps_sq = s1ps.tile([P, TPC], F32, name="ps_sumq", tag="sums",
                                      bufs=2)
                    ps_skv = s1ps.tile([P, TPC], F32, name="ps_sumkv", tag="sums",
                                       bufs=2)
                    fused_sb = {}
                    kpe_raw = s1.tile([cROPE, TPC], F32, name="kpe_raw")

                    def emit_group(gi, group):
                        g0 = chunk_cols(group[0])[0]
                        gw = sum(chunk_cols(fc)[1] for fc in group)
                        ps_f = {}
                        for dc in range(DCN):
                            ht = ht_sb[dc]
                            wt = s1.tile([P, GWMAX], S1MMDT,
                                         name=f"wfa_{gi}_{dc}", tag="wfa",
                                         bufs=2)
                            src_ap = wfa_d[dc * P:(dc + 1) * P, g0:g0 + gw]
                            if S1DT == F32:
                                src_ap = src_ap.bitcast(F32R)
                            nc.sync.dma_start(wt[:, :gw], src_ap)
                            off = 0
                            for fc in group:
                                cw = chunk_cols(fc)[1]
                                if dc == 0:
                                    ps_f[fc] = s1ps.tile([P, TPC], F32,
                                                         name=f"psf{fc}",
                                                         tag="fch", bufs=5)
                                nc.tensor.matmul(
                                    ps_f[fc][:cw, :], wt[:, off:off + cw],
                                    ht[:], start=(dc == 0),
                                    stop=(dc == DCN - 1))
                                off += cw
                        # drain this group's chunks
                        for fc in group:
                            cw = chunk_cols(fc)[1]
                            if fc < NFC:
                                ft = s1.tile([P, TPC], F32, name=f"fused{fc}")
                                nc.vector.tensor_copy(ft[:], ps_f[fc][:, :])
                                fused_sb[fc] = ft
                                x2 = s1.tile([P, TPC], F32R, name=f"x2_{fc}",
                                             tag="x2", bufs=4)
                                nc.vector.tensor_mul(x2[:], ft[:], ft[:])
                                tgt = ps_sq if fc < NQ else ps_skv
                                nc.tensor.matmul(
                                    tgt[:, :], ones_sq[:], x2[:],
                                    start=(fc == 0 or fc == NQ),
                                    stop=(fc == NQ - 1 or fc == NFC - 1))
                            else:
                                nc.vector.tensor_copy(kpe_raw[:], ps_f[fc][:cw, :])

                    for gi in range(len(groups)):
                        emit_group(gi, groups[gi])

                    rq = s1.tile([P, TPC], F32, name="rq")
                    rkv = s1.tile([P, TPC], F32, name="rkv")
                    sq_t = s1.tile([P, TPC], F32, name="sq_t", tag="sqt",
                                   bufs=2)
                    nc.scalar.activation(sq_t[:], ps_sq[:, :], AF.Sqrt,
                                         bias=eps_t[:], scale=1.0 / cQL)
                    nc.vector.reciprocal(rq[:], sq_t[:])
                    sq_t2 = s1.tile([P, TPC], F32, name="sq_t2", tag="sqt",
                                    bufs=2)
                    nc.scalar.activation(sq_t2[:], ps_skv[:, :], AF.Sqrt,
                                         bias=eps_t[:], scale=1.0 / cKVL)
                    nc.vector.reciprocal(rkv[:], sq_t2[:])
                    for fc in range(NFC):
                        rr = rq if fc < NQ else rkv
                        fb = s1.tile([P, TPC], BDT, name=f"fb{fc}", tag="fb",
                                     bufs=3)
                        nc.vector.tensor_mul(fb[:], fused_sb[fc][:], rr[:])
                        nc.sync.dma_start(ag1_in[fc * P:(fc + 1) * P, :],
                                          fb[:])
                    kpe_ro = s1.tile([cROPE, TPC], BDT, name="kpe_ro")
                    _rope_rotate(nc, s1, kpe_ro[:], kpe_raw[:], cosk_sb[:],
                                 sink_sb[:], TPC, "kpe")
                    nc.sync.dma_start(ag1_in[NFC * P:NFC * P + cROPE, :],
                                      kpe_ro[:])
                    if not no_cc:
                        nc.gpsimd.collective_compute(
                            "AllGather", mybir.AluOpType.bypass,
                            replica_groups=RG,
                            ins=[ag1_in.opt()], outs=[ag1_out.opt()])

                # ---------- phases B (per-head attention) and C (out proj) ----------
                with (
                    tc.tile_pool(name="pb", bufs=1) as pb,
                    tc.tile_pool(name="pbps", bufs=1, space="PSUM") as pbps,
                ):
                    kc_t = {}    # (tau, h) -> [128, TBLK] k_nope^T
                    kpe_t = {}   # tau -> [64, TBLK]
                    v_t = {}     # (tau, i) -> [128, HPC*128] natural v

                    def phase_b_block(b, tau):
                        tb = b * NT + tau
                        r0 = tb * NREG
                        # ---- rhs tiles for this block from the gathered latent ----
                        kvn = []
                        for cc in range(NKV):
                            t = pb.tile([P, TBLK], BMMDT, name=f"kvn{tb}_{cc}",
                                        tag="kvn", bufs=NKV)
                            for j in range(NREG):
                                src_ap = ag1_out[
                                    r0 + j, (NQ + cc) * P:(NQ + cc + 1) * P, :]
                                if BDT == F32:
                                    src_ap = src_ap.bitcast(F32R)
                                nc.sync.dma_start(
                                    t[:, j * TPC:(j + 1) * TPC], src_ap)
                            kvn.append(t)
                        kp = pb.tile([cROPE, TBLK], BMMDT, name=f"kpe{tb}",
                                     tag="kpt", bufs=NT)
                        for j in range(NREG):
                            src_ap = ag1_out[r0 + j,
                                             NFC * P:NFC * P + cROPE, :]
                            if BDT == F32:
                                src_ap = src_ap.bitcast(F32R)
                            nc.sync.dma_start(
                                kp[:, j * TPC:(j + 1) * TPC], src_ap)
                        kpe_t[tau] = kp

                        # ---- k_nope up-projection (transposed out) ----
                        for h in range(HPC):
                            ps = pbps.tile([P, TBLK], F32, name=f"psk{tb}_{h}",
                                           tag="acc", bufs=2)
                            for cc in range(NKV):
                                nc.tensor.matmul(
                                    ps[:, :],
                                    wkn_sb[cc][:, h * P:(h + 1) * P],
                                    kvn[cc][:], start=(cc == 0),
                                    stop=(cc == NKV - 1))
                            kt = pb.tile([P, TBLK], BMMDT, name=f"kc{tb}_{h}",
                                         tag="kc", bufs=NT * HPC)
                            nc.vector.tensor_copy(kt[:], ps[:, :])
                            kc_t[(tau, h)] = kt

                        # ---- v up-projection (natural layout) ----
                        for i in range(KB):
                            ps = pbps.tile([P, HPC * cVD], F32, name=f"psv{tb}_{i}",
                                           tag="acc", bufs=2)
                            for cc in range(NKV):
                                nc.tensor.matmul(
                                    ps[:, :],
                                    kvn[cc][:, i * P:(i + 1) * P],
                                    wv_sb[cc][:], start=(cc == 0),
                                    stop=(cc == NKV - 1))
                            vt = pb.tile([P, HPC * cVD], BMMDT, name=f"vt{tb}_{i}",
                                         tag="vt", bufs=NT * KB)
                            nc.vector.tensor_copy(vt[:], ps[:, :])
                            v_t[(tau, i)] = vt

                        # ---- q up-projection ----
                        qcn = []
                        for cc in range(NQ):
                            t = pb.tile([P, TBLK], BMMDT, name=f"qcn{tb}_{cc}",
                                        tag="qcn", bufs=NQ + 1)
                            for j in range(NREG):
                                src_ap = ag1_out[
                                    r0 + j, cc * P:(cc + 1) * P, :]
                                if BDT == F32:
                                    src_ap = src_ap.bitcast(F32R)
                                nc.sync.dma_start(
                                    t[:, j * TPC:(j + 1) * TPC], src_ap)
                            qcn.append(t)
                        qtn = []
                        for h in range(HPC):
                            ps = pbps.tile([P, TBLK], F32, name=f"psq{tb}_{h}",
                                           tag="acc", bufs=2)
                            for cc in range(NQ):
                                nc.tensor.matmul(
                                    ps[:, :],
                                    wqn_sb[cc][:, h * P:(h + 1) * P],
                                    qcn[cc][:], start=(cc == 0),
                                    stop=(cc == NQ - 1))
                            qt = pb.tile([P, TBLK], BMMDT, name=f"qtn{tb}_{h}",
                                         tag="qtn", bufs=HPC)
                            nc.vector.tensor_copy(qt[:], ps[:, :])
                            qtn.append(qt)
                        # rope part: output chunks hold up to 2 heads (64 rows each)
                        qtr = []
                        cos_sl = cosq_sb[:, tau * TBLK:(tau + 1) * TBLK]
                        sin_sl = sinq_sb[:, tau * TBLK:(tau + 1) * TBLK]
                        for oc in range(NROC):
                            nh = min(2, HPC - oc * 2)
                            rw = nh * cROPE
                            ps = pbps.tile([P, TBLK], F32, name=f"psr{tb}_{oc}",
                                           tag="acc", bufs=2)
                            for cc in range(NQ):
                                nc.tensor.matmul(
                                    ps[:rw, :],
                                    wqr_sb[cc][:, oc * P:oc * P + rw],
                                    qcn[cc][:], start=(cc == 0),
                                    stop=(cc == NQ - 1))
                            for s_ in range(nh):
                                h = oc * 2 + s_
                                qr = pb.tile([cROPE, TBLK], BMMDT,
                                             name=f"qtr{tb}_{h}", tag="qtr",
                                             bufs=HPC)
                                _rope_rotate(nc, pb, qr[:],
                                             ps[s_ * cROPE:(s_ + 1) * cROPE, :],
                                             cos_sl, sin_sl, TBLK, f"qr{tb}_{h}")
                                qtr.append(qr)

                        # ---- causal attention for this query block ----
                        nkc = (tau + 1) * KB
                        for h in range(HPC):
                            ps_den = pbps.tile([P, TBLK], F32, name=f"psd{tb}_{h}",
                                               tag="acc", bufs=2)
                            ps_ctx = pbps.tile([P, TBLK], F32, name=f"psc{tb}_{h}",
                                               tag="acc", bufs=2)
                            for kc in range(nkc):
                                tau_k, ik = kc // KB, kc % KB
                                ps_s = pbps.tile([P, TBLK], F32,
                                                 name=f"pss{tb}_{h}_{kc}",
                                                 tag="tr", bufs=2)
                                nc.tensor.matmul(
                                    ps_s[:, :],
                                    kc_t[(tau_k, h)][:, ik * P:(ik + 1) * P],
                                    qtn[h][:], start=True, stop=False)
                                nc.tensor.matmul(
                                    ps_s[:, :],
                                    kpe_t[tau_k][:, ik * P:(ik + 1) * P],
                                    qtr[h][:], start=False, stop=True)
                                ex = pb.tile([P, TBLK], BMMDT,
                                             name=f"ex{tb}_{h}_{kc}", tag="ex",
                                             bufs=2)
                                nc.scalar.activation(ex[:], ps_s[:, :], AF.Exp,
                                                     scale=SCALE)
                                m = kc - tau * KB
                                if m >= 0:
                                    nc.vector.tensor_mul(ex[:], ex[:],
                                                         masks_sb[m][:])
                                nc.tensor.matmul(ps_den[:, :], ones_b[:],
                                                 ex[:], start=(kc == 0),
                                                 stop=(kc == nkc - 1))
                                nc.tensor.matmul(
                                    ps_ctx[:, :],
                                    v_t[(tau_k, ik)][:, h * P:(h + 1) * P],
                                    ex[:], start=(kc == 0),
                                    stop=(kc == nkc - 1))
                            rec = pb.tile([P, TBLK], F32, name=f"rec{tb}_{h}",
                                          tag="rec", bufs=1)
                            nc.vector.reciprocal(rec[:], ps_den[:, :])
                            cx = pb.tile([P, TBLK], CDT, name=f"cx{tb}_{h}",
                                         tag="cx", bufs=2)
                            nc.vector.tensor_mul(cx[:], ps_ctx[:, :], rec[:])
                            nc.sync.dma_start(
                                ag2_in[(b, tau)][h * P:(h + 1) * P, :], cx[:])

                        if not no_cc:
                            nc.gpsimd.collective_compute(
                                "AllGather", mybir.AluOpType.bypass,
                                replica_groups=RG,
                                ins=[ag2_in[(b, tau)].opt()],
                                outs=[ag2_out[(b, tau)].opt()])

                    def phase_c_block(b, tau):
                        tb = b * NT + tau
                        nec = DPC // P
                        ps_o = [pbps.tile([P, TBLK], F32, name=f"pso{tb}_{e}",
                                          tag="co", bufs=DPC // P)
                                for e in range(nec)]
                        for hv in range(cH * cVD // P):
                            ct = pb.tile([P, TBLK], CMMDT,
                                         name=f"ct{tb}_{hv}", tag="ct", bufs=3)
                            src_ap = ag2_out[(b, tau)][
                                hv // HPC, (hv % HPC) * P:(hv % HPC + 1) * P, :]
                            if CDT == F32:
                                src_ap = src_ap.bitcast(F32R)
                            nc.sync.dma_start(ct[:], src_ap)
                            wt = pb.tile([P, DPC], CMMDT,
                                         name=f"wor{tb}_{hv}", tag="wor",
                                         bufs=3)
                            src_ap = wo_d[hv * P:(hv + 1) * P, :]
                            if CDT == F32:
                                src_ap = src_ap.bitcast(F32R)
                            nc.sync.dma_start(wt[:], src_ap)
                            for e in range(nec):
                                nc.tensor.matmul(
                                    ps_o[e][:, :], wt[:, e * P:(e + 1) * P],
                                    ct[:], start=(hv == 0),
                                    stop=(hv == cH * cVD // P - 1))
                        for e in range(nec):
                            ot = pb.tile([P, TBLK], F32, name=f"ot{tb}_{e}",
                                         tag="ot", bufs=2)
                            nc.vector.tensor_copy(ot[:], ps_o[e][:, :])
                            nc.sync.dma_start(
                                outT_d[e * P:(e + 1) * P,
                                       b * cS + tau * TBLK:
                                       b * cS + (tau + 1) * TBLK],
                                ot[:])

                    # emission order interleaves C into B for PE overlap with AG2
                    blocks = [(b, t_) for b in range(cB) for t_ in range(NT)]
                    done_c = 0
                    for i, (b, t_) in enumerate(blocks):
                        phase_b_block(b, t_)
                        if t_ == NT - 1:
                            kc_t.clear()
                            kpe_t.clear()
                            v_t.clear()
                        # after at least 2 B-blocks are out, start interleaving C
                        if i >= 2:
                            phase_c_block(*blocks[done_c])
                            done_c += 1
                    while done_c < len(blocks):
                        phase_c_block(*blocks[done_c])
                        done_c += 1


            for _rep in range(reps):
                one_rep()
                if tiny_out:
                    tk = cst.tile([P, 4], F32, name="tick_sb", tag="tick",
                                  bufs=2)
                    for jj in range(4):
                        nc.sync.dma_start(
                            tk[:, jj:jj + 1],
                            outT_d[0:P, jj * (T // 4):jj * (T // 4) + 1])
                    nc.sync.dma_start(tick_d[:, :], tk[:])

    nc.compile()
    return nc


# ---------------- host wrapper ----------------

def _host_prep(inputs, cfg=None):
    c = dict(DEFAULT_CFG)
    if cfg:
        c.update(cfg)
    cB, cS, cD = c["B"], c["S"], c["D"]
    cQL, cKVL, cH = c["QL"], c["KVL"], c["H"]
    cNOPE, cROPE, cVD, cNC = c["NOPE"], c["ROPE"], c["VD"], c["NC"]
    cQKD = cNOPE + cROPE
    P = 128
    T = cB * cS
    TPC = T // cNC
    HPC = cH // cNC
    DPC = cD // cNC
    TBLK = min(512, cS)
    KB = TBLK // P

    import ml_dtypes
    s1dt = ml_dtypes.bfloat16 if c["bf16_s1"] else np.float32
    cdt = ml_dtypes.bfloat16 if c["bf16_c"] else np.float32
    bdt = ml_dtypes.bfloat16 if c["bf16_b"] else np.float32
    hs = np.ascontiguousarray(np.asarray(inputs["hidden_states"], np.float32))
    wfa = np.ascontiguousarray(
        np.asarray(inputs["w_fused_a"], np.float32).astype(s1dt))
    gq = np.asarray(inputs["q_a_ln_w"], np.float32)
    gkv = np.asarray(inputs["kv_a_ln_w"], np.float32)
    wqb = np.asarray(inputs["w_q_b"], np.float32)
    wkvb = np.asarray(inputs["w_kv_b"], np.float32)
    wo = np.asarray(inputs["w_o"], np.float32)

    hflat = hs.reshape(T, cD)
    wq = (gq[:, None] * wqb).reshape(cQL, cH, cQKD)
    wkv = (gkv[:, None] * wkvb).reshape(cKVL, cH, cNOPE + cVD)

    half = cROPE // 2
    inv_freq = (1.0 / (np.float32(ROPE_BASE) **
                       (np.arange(half, dtype=np.float32) / np.float32(half))))
    ang = (np.arange(cS, dtype=np.float32)[:, None]
           * inv_freq[None, :].astype(np.float32))
    cosT = np.ascontiguousarray(np.cos(ang).astype(np.float32).T)  # [half, S]
    sinT = np.ascontiguousarray(np.sin(ang).astype(np.float32).T)

    ii = np.arange(P)[:, None, None]
    mm = np.arange(KB)[None, :, None]
    jj = np.arange(TBLK)[None, None, :]
    masks = ((ii + P * mm) <= jj).astype(np.float32)

    in_maps = []
    for rk in range(cNC):
        tok = slice(rk * TPC, (rk + 1) * TPC)
        hd = slice(rk * HPC, (rk + 1) * HPC)
        s0 = (rk * TPC) % cS
        in_maps.append({
            "hiddenT": np.ascontiguousarray(hflat[tok].T).astype(s1dt),
            "wfa": wfa,
            "wqn": np.ascontiguousarray(
                wq[:, hd, :cNOPE].reshape(cQL, HPC * cNOPE)).astype(bdt),
            "wqr": np.ascontiguousarray(
                wq[:, hd, cNOPE:].reshape(cQL, HPC * cROPE)).astype(bdt),
            "wkn": np.ascontiguousarray(
                wkv[:, hd, :cNOPE].reshape(cKVL, HPC * cNOPE)).astype(bdt),
            "wv": np.ascontiguousarray(
                wkv[:, hd, cNOPE:].reshape(cKVL, HPC * cVD)).astype(bdt),
            "wo": np.ascontiguousarray(
                wo[:, rk * DPC:(rk + 1) * DPC]).astype(cdt),
            "cosq": cosT,
            "sinq": sinT,
            "cosk": np.ascontiguousarray(cosT[:, s0:s0 + TPC]),
            "sink": np.ascontiguousarray(sinT[:, s0:s0 + TPC]),
            "masks": masks.astype(bdt),
            "ones": np.ones((P, P), np.float32),
            "onesb": np.ones((P, P), ml_dtypes.bfloat16),
        })
    return in_maps


def _assemble(results, cfg=None):
    c = dict(DEFAULT_CFG)
    if cfg:
        c.update(cfg)
    cB, cS, cD = c["B"], c["S"], c["D"]
    outT = np.concatenate([r["outT"] for r in results], axis=0)  # [D, T]
    return np.ascontiguousarray(outT.T).reshape(cB, cS, cD).astype(np.float32)


def kernel(**inputs):
    nc = build_program()
    in_maps = _host_prep(inputs)
    res = run_bass_kernel_spmd(nc, in_maps, list(range(NC)))
    return _assemble(res.results)

